# revision 20
# baseline (speedup 1.0000x reference)
"""Causal self-attention with RoPE on 8 Trainium2 NeuronCores.

Problem (fixed shapes): x (4, 2048, 1024) f32, Wqkv (1024, 3072), Wout
(1024, 1024), causal mask.  16 heads, head_dim 64, rope base 10000.

Sharding: 8 cores = 4 batches x 2 head-groups (8 heads each).  Each core:
  qT/kT = (Wq_shard).T-projection of x_b (computed directly transposed:
      psum[j, s] = sum_d Wq[d, j] * xT[d, s]), RoPE applied on-chip,
  v    = natural-layout projection, augmented with a ones column so the
         PV matmul also produces softmax denominators,
  attention in scores-transposed layout (kv on partitions, q on free):
      scoresT = kT_slice.T @ qT_slice, exp on ACT (scale=1/8 folded),
      causal masking via precomputed 0/1 tiles, PV accumulates
      ctxT (65 x 512) in PSUM per (head, q-block),
  out_partial = ctxT @ Wout_shard  -> (2048, 1024) f32 per core.
Host sums the two head-group partials per batch.

All matmuls bf16 (f32 PSUM accumulation).
"""
import os
import sys

for _p in ("/opt/trn_rl_repo", "/root/.axon_site/_ro/trn_rl_repo"):
    if os.path.isdir(_p) and _p not in sys.path:
        sys.path.insert(0, _p)

import numpy as np
import ml_dtypes
from contextlib import ExitStack

BF = ml_dtypes.bfloat16

P = 128          # sbuf partitions
S = 2048         # sequence length
D = 1024         # model dim
HD = 64          # head dim
NH = 8           # heads per core
DQ = 512         # per-core projection width (NH * HD)
ND = D // P      # 8 contraction chunks for the qkv projection
NJ = DQ // P     # 4 feature tiles of qT/kT/ctxT
QB = 512         # q block width
NQB = S // QB    # 4
NKV = S // P     # 16 kv tiles
VW = HD + 1      # v block width incl. ones column (65)
ROPE_BASE = 10000.0
NCORES = 8

_CACHE = {}


def _build_nc(loop_n=None, parts="bcd"):
    """loop_n: wrap the whole body in a hardware For loop (timing only).
    parts: which phases to emit (timing bisection only)."""
    import contextlib
    import concourse.tile as tile
    from concourse import bacc, mybir

    f32 = mybir.dt.float32
    bf16 = mybir.dt.bfloat16
    Exp = mybir.ActivationFunctionType.Exp

    nc = bacc.Bacc("TRN2", target_bir_lowering=False, debug=False,
                   num_devices=NCORES)

    # Host pre-chunks everything so all loads are contiguous (p, free).
    xT_d = nc.dram_tensor("xT", [P, ND * S], bf16, kind="ExternalInput").ap()
    wq_d = nc.dram_tensor("wq", [P, ND * DQ], bf16, kind="ExternalInput").ap()
    wk_d = nc.dram_tensor("wk", [P, ND * DQ], bf16, kind="ExternalInput").ap()
    wv_d = nc.dram_tensor("wv", [P, ND * DQ], bf16, kind="ExternalInput").ap()
    wo_d = nc.dram_tensor("wo", [P, NJ * D], bf16, kind="ExternalInput").ap()
    cos_d = nc.dram_tensor("cosT", [P, S], bf16, kind="ExternalInput").ap()
    sin_d = nc.dram_tensor("sinT", [P, S], bf16, kind="ExternalInput").ap()
    msk_d = nc.dram_tensor("msk", [P, 2 * P], bf16, kind="ExternalInput").ap()
    rm_d = nc.dram_tensor("rm", [P, P], bf16, kind="ExternalInput").ap()
    out_d = nc.dram_tensor("outp", [S, D], f32, kind="ExternalOutput").ap()

    with tile.TileContext(nc) as tc, ExitStack() as ctx:
        pers = ctx.enter_context(tc.tile_pool(name="pers", bufs=1))
        work = ctx.enter_context(tc.tile_pool(name="work", bufs=2))
        apool = ctx.enter_context(tc.tile_pool(name="apool", bufs=4))
        opool = ctx.enter_context(tc.tile_pool(name="opool", bufs=4))
        ppool = ctx.enter_context(tc.tile_pool(name="pp", bufs=2, space="PSUM"))
        spool = ctx.enter_context(tc.tile_pool(name="sp", bufs=2, space="PSUM"))
        cpool = ctx.enter_context(tc.tile_pool(name="cp", bufs=2, space="PSUM"))

        xT = pers.tile([P, ND * S], bf16, tag="xT", name="xT_sb")
        wq = pers.tile([P, ND * DQ], bf16, tag="wq", name="wq_sb")
        wk = pers.tile([P, ND * DQ], bf16, tag="wk", name="wk_sb")
        wv = pers.tile([P, ND * DQ], bf16, tag="wv", name="wv_sb")
        wo = pers.tile([P, NJ * D], bf16, tag="wo", name="wo_sb")
        cosb = pers.tile([P, S], bf16, tag="cos", name="cos_sb")
        sinb = pers.tile([P, S], bf16, tag="sin", name="sin_sb")
        msk = pers.tile([P, 2 * P], bf16, tag="msk", name="msk_sb")
        rm = pers.tile([P, P], bf16, tag="rm", name="rm_sb")
        qT = pers.tile([P, NJ * S], bf16, tag="qT", name="qT_sb")
        kT = pers.tile([P, NJ * S], bf16, tag="kT", name="kT_sb")
        vsb = pers.tile([P, NH * NKV * VW], bf16, tag="v", name="v_sb")
        cx = pers.tile([P, NJ * S], bf16, tag="cx", name="cx_sb")

        loop_cm = tc.For_i(0, loop_n, 1) if loop_n else contextlib.nullcontext()
        loop_cm.__enter__()

        # chunked loads so the d-loop of the first projections can start
        # as soon as chunk 0 lands
        for d in range(ND):
            nc.sync.dma_start(wq[:, d * DQ:(d + 1) * DQ],
                              wq_d[:, d * DQ:(d + 1) * DQ])
            nc.sync.dma_start(xT[:, d * S:(d + 1) * S],
                              xT_d[:, d * S:(d + 1) * S])
            nc.sync.dma_start(wk[:, d * DQ:(d + 1) * DQ],
                              wk_d[:, d * DQ:(d + 1) * DQ])
            nc.sync.dma_start(wv[:, d * DQ:(d + 1) * DQ],
                              wv_d[:, d * DQ:(d + 1) * DQ])
            if d == 0:
                nc.sync.dma_start(rm[:], rm_d)
                nc.sync.dma_start(cosb[:], cos_d)
                nc.sync.dma_start(sinb[:], sin_d)
        nc.sync.dma_start(msk[:], msk_d)
        nc.sync.dma_start(wo[:], wo_d)

        # ones column of the augmented v blocks: cols 64, 129, 194, ...
        nc.vector.memset(vsb[:, HD::VW], 1.0)

        # zero both scores-psum slots once: causal-trimmed score matmuls
        # leave sub-diagonal columns unwritten, and exp(garbage psum) could
        # be inf (inf * 0 mask -> NaN).  Bounded leftovers are fine.
        z0 = spool.tile([P, 2 * QB], mybir.dt.float32, tag="sp", name="z0")
        z1 = spool.tile([P, 2 * QB], mybir.dt.float32, tag="sp", name="z1")
        nc.vector.memset(z0[:], 0.0)
        nc.vector.memset(z1[:], 0.0)

        # ---- Phase B: q/k/v projections (interleaved) + RoPE ----------
        vv = vsb.rearrange("p (h t c) -> p h t c", h=NH, t=NKV, c=VW)

        def proj_qk_mm(w_sb, j, sb):
            qps = ppool.tile([P, QB], f32, tag="pp", name="qps")
            for d in range(ND):
                nc.tensor.matmul(
                    qps[:],
                    w_sb[:, d * DQ + j * P: d * DQ + (j + 1) * P],
                    xT[:, d * S + sb * QB: d * S + (sb + 1) * QB],
                    start=(d == 0), stop=(d == ND - 1))
            qc = work.tile([P, QB], bf16, tag="qc", name="qc")
            nc.vector.tensor_copy(qc[:], qps[:])
            return qc

        def rope_tail(qc, dst, j, sb):
            # rotate_half as a constant +-1 matmul (PE); psum slot from spool
            rps = spool.tile([P, QB], f32, tag="sp", name="rps")
            nc.tensor.matmul(rps[:], rm[:], qc[:], start=True, stop=True)
            q1 = work.tile([P, QB], bf16, tag="q1", name="q1")
            nc.vector.tensor_mul(q1[:], qc[:], cosb[:, sb * QB:(sb + 1) * QB])
            q2 = work.tile([P, QB], bf16, tag="q2", name="q2")
            nc.vector.tensor_mul(q2[:], rps[:], sinb[:, sb * QB:(sb + 1) * QB])
            nc.vector.tensor_add(
                dst[:, j * S + sb * QB: j * S + (sb + 1) * QB], q1[:], q2[:])

        def proj_v(st):
            vps = ppool.tile([P, DQ], f32, tag="pp", name="vps")
            for d in range(ND):
                nc.tensor.matmul(
                    vps[:],
                    xT[:, d * S + st * P: d * S + (st + 1) * P],
                    wv[:, d * DQ: (d + 1) * DQ],
                    start=(d == 0), stop=(d == ND - 1))
            nc.scalar.copy(vv[:, :, st, 0:HD],
                           vps.rearrange("p (h c) -> p h c", h=NH))

        # emit the rotate matmul a phase late so PE never waits on the
        # DVE psum->sbuf copy feeding it
        for i in range(NKV):
            j, sb = divmod(i, NQB)
            qc_q = proj_qk_mm(wq, j, sb)
            qc_k = proj_qk_mm(wk, j, sb)
            rope_tail(qc_q, qT, j, sb)
            proj_v(i)
            rope_tail(qc_k, kT, j, sb)

        # ---- Phase C: attention --------------------------------------
        if "c" not in parts:
            ctx2 = None
        # head PAIRS: heads (2*jh, 2*jh+1) live in rows [0:64] / [64:128]
        # of kT/qT j-tile jh.  Their score matmuls (K=64) run concurrently
        # in disjoint PE row groups (tile_position from base_partition),
        # writing the two halves [A | B] of one psum tile per kv-tile.
        for jh in range(NJ if "c" in parts else 0):
            for jq in range(NQB):
                cpsA = cpool.tile([VW, QB], f32, tag="cp", name="cpsA")
                cpsB = cpool.tile([VW, QB], f32, tag="cp", name="cpsB")
                nt = 4 * (jq + 1)          # valid kv tiles

                def pv(t, att):
                    dq = max(0, t * P - jq * QB)
                    for rh, cps in ((0, cpsA), (1, cpsB)):
                        h = 2 * jh + rh
                        nc.tensor.matmul(
                            cps[:, dq:QB],
                            vsb[:, (h * NKV + t) * VW: (h * NKV + t + 1) * VW],
                            att[:, rh * QB + dq:(rh + 1) * QB],
                            start=(t == 0),
                            stop=(t == nt - 1),
                            skip_group_check=True)

                pending = []
                for t in range(nt):
                    sps = spool.tile([P, 2 * QB], f32, tag="sp", name="sps")
                    att = apool.tile([P, 2 * QB], bf16, tag="att", name="att")
                    dq = max(0, t * P - jq * QB)  # causal left-trim
                    for rh in range(2):
                        r0 = rh * HD
                        nc.tensor.matmul(
                            sps[:, rh * QB + dq:(rh + 1) * QB],
                            kT[r0:r0 + HD, jh * S + t * P: jh * S + (t + 1) * P],
                            qT[r0:r0 + HD,
                               jh * S + jq * QB + dq: jh * S + (jq + 1) * QB],
                            start=True, stop=True)
                    nc.scalar.activation(att[:], sps[:], Exp, scale=0.125)
                    if t >= 4 * jq:
                        # zero the upper triangle of the diagonal 128x128
                        # block in both halves: one op, free dims (2, 128)
                        attv = att.rearrange("p (h c) -> p h c", h=2)
                        nc.vector.tensor_mul(
                            attv[:, :, dq:dq + P], attv[:, :, dq:dq + P],
                            msk.rearrange("p (h c) -> p h c", h=2))
                    # PV runs 2 kv-tiles behind scores so PE never waits on
                    # exp (ACT) + mask (DVE) latency
                    pending.append((t, att))
                    if len(pending) > 2:
                        pv(*pending.pop(0))
                for pe_args in pending:
                    pv(*pe_args)
                # normalize: ctxT = cps[0:64] * (1 / denom row), write into cx
                for rh, cps in ((0, cpsA), (1, cpsB)):
                    r0 = rh * HD
                    rec1 = work.tile([1, QB], mybir.dt.float32, tag="rec", name="rec1")
                    nc.vector.reciprocal(rec1[:], cps[HD:HD + 1, :])
                    recb = work.tile([HD, QB], mybir.dt.float32, tag="recb", name="recb")
                    nc.gpsimd.partition_broadcast(recb[:], rec1[:])
                    nc.vector.tensor_mul(
                        cx[r0:r0 + HD, jh * S + jq * QB: jh * S + (jq + 1) * QB],
                        cps[0:HD, :], recb[:])

        # ---- Phase D: out projection ---------------------------------
        # round-robin psum over both pools (sp slots are free now) and
        # alternate the psum->sbuf copy between DVE and ACT so PE never
        # waits on a drain
        for st in range(NKV if "d" in parts else 0):
            for eb in range(2):
                i = st * 2 + eb
                if i % 2 == 0:
                    ops = ppool.tile([P, QB], f32, tag="pp", name="ops")
                else:
                    ops = spool.tile([P, QB], f32, tag="sp", name="ops2")
                for fc in range(NJ):
                    nc.tensor.matmul(
                        ops[:],
                        cx[:, fc * S + st * P: fc * S + (st + 1) * P],
                        wo[:, fc * D + eb * QB: fc * D + (eb + 1) * QB],
                        start=(fc == 0), stop=(fc == NJ - 1))
                ob = opool.tile([P, QB], f32, tag="ob", name="ob")
                if i % 2 == 0:
                    nc.vector.tensor_copy(ob[:], ops[:])
                else:
                    nc.scalar.copy(ob[:], ops[:])
                nc.sync.dma_start(
                    out_d[st * P:(st + 1) * P, eb * QB:(eb + 1) * QB], ob[:])

        loop_cm.__exit__(None, None, None)

    nc.compile()
    return nc


def _chunk_rows(a, np_rows):
    """(n*P, F) -> (P, n*F) with chunk i of rows at free offset i*F."""
    n = a.shape[0] // np_rows
    return np.ascontiguousarray(
        a.reshape(n, np_rows, a.shape[1]).transpose(1, 0, 2).reshape(np_rows, -1))


def _host_tables():
    inv = 1.0 / (ROPE_BASE ** (np.arange(0, HD, 2, dtype=np.float64) / HD))
    t = np.arange(S, dtype=np.float64)
    fr = np.outer(inv, t)                      # (32, S)
    cosw = np.cos(fr)
    sinw = np.sin(fr)
    cosT = np.tile(cosw, (4, 1))               # rows: 4 x 32 pattern
    sinT = np.tile(sinw, (4, 1))               # sign lives in rmat
    # 0/1 upper-triangle (keep q >= kv) mask for diagonal blocks, doubled
    r = np.arange(P)[:, None]
    c = np.arange(P)[None, :]
    tri = (c >= r).astype(np.float32)
    m = np.concatenate([tri, tri], axis=1)
    # rotate-half matrix: out[p] = sum_r rm[r, p] * in[r]
    rmat = np.zeros((P, P), np.float32)
    for b in range(2):
        for i in range(32):
            p = b * 64 + i
            rmat[p + 32, p] = -1.0             # out[p] = -in[p+32]
            rmat[p, p + 32] = 1.0              # out[p+32] = in[p]
    return (cosT.astype(BF), sinT.astype(BF), m.astype(BF), rmat.astype(BF))


def kernel(x, Wqkv, Wout, mask=None, **_unused):
    x = np.asarray(x, dtype=np.float32)
    Wqkv = np.asarray(Wqkv, dtype=np.float32)
    Wout = np.asarray(Wout, dtype=np.float32)
    B = x.shape[0]

    cosT, sinT, mskv, rmat = _host_tables()

    in_maps = []
    for core in range(NCORES):
        b, hg = core // 2, core % 2
        sl = slice(hg * DQ, hg * DQ + DQ)
        xT = np.ascontiguousarray(x[b].T).astype(BF)       # (1024, 2048)
        in_maps.append({
            "xT": _chunk_rows(xT, P),
            "wq": _chunk_rows(Wqkv[:, 0 * D + hg * DQ: 0 * D + hg * DQ + DQ].astype(BF), P),
            "wk": _chunk_rows(Wqkv[:, 1 * D + hg * DQ: 1 * D + hg * DQ + DQ].astype(BF), P),
            "wv": _chunk_rows(Wqkv[:, 2 * D + hg * DQ: 2 * D + hg * DQ + DQ].astype(BF), P),
            "wo": _chunk_rows(Wout[sl, :].astype(BF), P),
            "cosT": cosT, "sinT": sinT, "msk": mskv, "rm": rmat,
        })

    if "nc" not in _CACHE:
        _CACHE["nc"] = _build_nc()
    nc = _CACHE["nc"]

    from concourse.bass_utils import run_bass_kernel_spmd
    res = run_bass_kernel_spmd(nc, in_maps, list(range(NCORES))).results

    out = np.zeros((B, S, D), np.float32)
    for core in range(NCORES):
        out[core // 2] += res[core]["outp"]
    return out


# revision 21
# speedup vs baseline: 1.4574x; 1.4574x over previous
"""Causal self-attention with RoPE on 8 Trainium2 NeuronCores.

Problem (fixed shapes): x (4, 2048, 1024) f32, Wqkv (1024, 3072), Wout
(1024, 1024), causal mask.  16 heads, head_dim 64, rope base 10000.

Sharding: 8 cores = 4 batches x 2 head-groups (8 heads each).  Each core:
  qT/kT = (Wq_shard).T-projection of x_b (computed directly transposed:
      psum[j, s] = sum_d Wq[d, j] * xT[d, s]), RoPE applied on-chip,
  v    = natural-layout projection, augmented with a ones column so the
         PV matmul also produces softmax denominators,
  attention in scores-transposed layout (kv on partitions, q on free):
      scoresT = kT_slice.T @ qT_slice, exp on ACT (scale=1/8 folded),
      causal masking via precomputed 0/1 tiles, PV accumulates
      ctxT (65 x 512) in PSUM per (head, q-block),
  out_partial = ctxT @ Wout_shard  -> (2048, 1024) f32 per core.
Host sums the two head-group partials per batch.

All matmuls bf16 (f32 PSUM accumulation).
"""
import os
import sys

for _p in ("/opt/trn_rl_repo", "/root/.axon_site/_ro/trn_rl_repo"):
    if os.path.isdir(_p) and _p not in sys.path:
        sys.path.insert(0, _p)

import numpy as np
import ml_dtypes
from contextlib import ExitStack

BF = ml_dtypes.bfloat16

P = 128          # sbuf partitions
S = 2048         # sequence length
D = 1024         # model dim
HD = 64          # head dim
NH = 8           # heads per core
DQ = 512         # per-core projection width (NH * HD)
ND = D // P      # 8 contraction chunks for the qkv projection
NJ = DQ // P     # 4 feature tiles of qT/kT/ctxT
QB = 512         # q block width
NQB = S // QB    # 4
NKV = S // P     # 16 kv tiles
VW = HD + 1      # v block width incl. ones column (65)
ROPE_BASE = 10000.0
NCORES = 8

_CACHE = {}


def _build_nc(loop_n=None, parts="bcd"):
    """loop_n: wrap the whole body in a hardware For loop (timing only).
    parts: which phases to emit (timing bisection only)."""
    import contextlib
    import concourse.tile as tile
    from concourse import bacc, mybir

    f32 = mybir.dt.float32
    bf16 = mybir.dt.bfloat16
    Exp = mybir.ActivationFunctionType.Exp

    nc = bacc.Bacc("TRN2", target_bir_lowering=False, debug=False,
                   num_devices=NCORES)

    # Host pre-chunks everything so all loads are contiguous (p, free).
    xT_d = nc.dram_tensor("xT", [P, ND * S], bf16, kind="ExternalInput").ap()
    wq_d = nc.dram_tensor("wq", [P, ND * DQ], bf16, kind="ExternalInput").ap()
    wk_d = nc.dram_tensor("wk", [P, ND * DQ], bf16, kind="ExternalInput").ap()
    wv_d = nc.dram_tensor("wv", [P, ND * DQ], bf16, kind="ExternalInput").ap()
    wo_d = nc.dram_tensor("wo", [P, NJ * D], bf16, kind="ExternalInput").ap()
    cos_d = nc.dram_tensor("cosT", [P, S], bf16, kind="ExternalInput").ap()
    sin_d = nc.dram_tensor("sinT", [P, S], bf16, kind="ExternalInput").ap()
    msk_d = nc.dram_tensor("msk", [P, 2 * P], bf16, kind="ExternalInput").ap()
    rm_d = nc.dram_tensor("rm", [P, P], bf16, kind="ExternalInput").ap()
    out_d = nc.dram_tensor("outp", [S, D], f32, kind="ExternalOutput").ap()

    with tile.TileContext(nc) as tc, ExitStack() as ctx:
        pers = ctx.enter_context(tc.tile_pool(name="pers", bufs=1))
        work = ctx.enter_context(tc.tile_pool(name="work", bufs=2))
        apool = ctx.enter_context(tc.tile_pool(name="apool", bufs=4))
        opool = ctx.enter_context(tc.tile_pool(name="opool", bufs=4))
        # one 4-slot pool of 1-bank tiles shared by all phases (proj psum,
        # ctx accumulators, out-proj psum) + one 2-slot pool of 2-bank tiles
        # (scores, rotate, out-proj alt) = 8 banks exactly
        bpool = ctx.enter_context(tc.tile_pool(name="b1", bufs=4, space="PSUM"))
        spool = ctx.enter_context(tc.tile_pool(name="sp", bufs=2, space="PSUM"))

        xT = pers.tile([P, ND * S], bf16, tag="xT", name="xT_sb")
        wq = pers.tile([P, ND * DQ], bf16, tag="wq", name="wq_sb")
        wk = pers.tile([P, ND * DQ], bf16, tag="wk", name="wk_sb")
        wv = pers.tile([P, ND * DQ], bf16, tag="wv", name="wv_sb")
        wo = pers.tile([P, NJ * D], bf16, tag="wo", name="wo_sb")
        cosb = pers.tile([P, S], bf16, tag="cos", name="cos_sb")
        sinb = pers.tile([P, S], bf16, tag="sin", name="sin_sb")
        msk = pers.tile([P, 2 * P], bf16, tag="msk", name="msk_sb")
        rm = pers.tile([P, P], bf16, tag="rm", name="rm_sb")
        qT = pers.tile([P, NJ * S], bf16, tag="qT", name="qT_sb")
        kT = pers.tile([P, NJ * S], bf16, tag="kT", name="kT_sb")
        vsb = pers.tile([P, NH * NKV * VW], bf16, tag="v", name="v_sb")
        cx = pers.tile([P, NJ * S], bf16, tag="cx", name="cx_sb")

        loop_cm = tc.For_i(0, loop_n, 1) if loop_n else contextlib.nullcontext()
        loop_cm.__enter__()

        # chunked loads so the d-loop of the first projections can start
        # as soon as chunk 0 lands
        for d in range(ND):
            nc.sync.dma_start(wq[:, d * DQ:(d + 1) * DQ],
                              wq_d[:, d * DQ:(d + 1) * DQ])
            nc.sync.dma_start(xT[:, d * S:(d + 1) * S],
                              xT_d[:, d * S:(d + 1) * S])
            nc.sync.dma_start(wk[:, d * DQ:(d + 1) * DQ],
                              wk_d[:, d * DQ:(d + 1) * DQ])
            nc.sync.dma_start(wv[:, d * DQ:(d + 1) * DQ],
                              wv_d[:, d * DQ:(d + 1) * DQ])
            if d == 0:
                nc.sync.dma_start(rm[:], rm_d)
                nc.sync.dma_start(cosb[:], cos_d)
                nc.sync.dma_start(sinb[:], sin_d)
        nc.sync.dma_start(msk[:], msk_d)
        nc.sync.dma_start(wo[:], wo_d)

        # ones column of the augmented v blocks: cols 64, 129, 194, ...
        nc.vector.memset(vsb[:, HD::VW], 1.0)

        # zero both scores-psum slots once: causal-trimmed score matmuls
        # leave sub-diagonal columns unwritten, and exp(garbage psum) could
        # be inf (inf * 0 mask -> NaN).  Bounded leftovers are fine.
        z0 = spool.tile([P, 2 * QB], mybir.dt.float32, tag="sp", name="z0")
        z1 = spool.tile([P, 2 * QB], mybir.dt.float32, tag="sp", name="z1")
        nc.vector.memset(z0[:], 0.0)
        nc.vector.memset(z1[:], 0.0)

        # ---- Phase B: q/k/v projections (interleaved) + RoPE ----------
        vv = vsb.rearrange("p (h t c) -> p h t c", h=NH, t=NKV, c=VW)

        def proj_qk_mm(w_sb, j, sb):
            qps = bpool.tile([P, QB], f32, tag="b1", name="qps")
            for d in range(ND):
                nc.tensor.matmul(
                    qps[:],
                    w_sb[:, d * DQ + j * P: d * DQ + (j + 1) * P],
                    xT[:, d * S + sb * QB: d * S + (sb + 1) * QB],
                    start=(d == 0), stop=(d == ND - 1))
            qc = work.tile([P, QB], bf16, tag="qc", name="qc")
            nc.vector.tensor_copy(qc[:], qps[:])
            return qc

        def rope_tail(qc, dst, j, sb):
            # rotate_half as a constant +-1 matmul (PE); psum slot from spool
            rps = spool.tile([P, QB], f32, tag="sp", name="rps")
            nc.tensor.matmul(rps[:], rm[:], qc[:], start=True, stop=True)
            q1 = work.tile([P, QB], bf16, tag="q1", name="q1")
            nc.vector.tensor_mul(q1[:], qc[:], cosb[:, sb * QB:(sb + 1) * QB])
            q2 = work.tile([P, QB], bf16, tag="q2", name="q2")
            nc.vector.tensor_mul(q2[:], rps[:], sinb[:, sb * QB:(sb + 1) * QB])
            nc.vector.tensor_add(
                dst[:, j * S + sb * QB: j * S + (sb + 1) * QB], q1[:], q2[:])

        def proj_v(st):
            vps = bpool.tile([P, DQ], f32, tag="b1", name="vps")
            for d in range(ND):
                nc.tensor.matmul(
                    vps[:],
                    xT[:, d * S + st * P: d * S + (st + 1) * P],
                    wv[:, d * DQ: (d + 1) * DQ],
                    start=(d == 0), stop=(d == ND - 1))
            nc.scalar.copy(vv[:, :, st, 0:HD],
                           vps.rearrange("p (h c) -> p h c", h=NH))

        # emit the rotate matmul a phase late so PE never waits on the
        # DVE psum->sbuf copy feeding it
        for i in range(NKV):
            j, sb = divmod(i, NQB)
            qc_q = proj_qk_mm(wq, j, sb)
            qc_k = proj_qk_mm(wk, j, sb)
            rope_tail(qc_q, qT, j, sb)
            proj_v(i)
            rope_tail(qc_k, kT, j, sb)

        # ---- Phase C: attention --------------------------------------
        if "c" not in parts:
            ctx2 = None
        # head PAIRS: heads (2*jh, 2*jh+1) live in rows [0:64] / [64:128]
        # of kT/qT j-tile jh.  Their score matmuls (K=64) run concurrently
        # in disjoint PE row groups (tile_position from base_partition),
        # writing the two halves [A | B] of one psum tile per kv-tile.
        for jh in range(NJ if "c" in parts else 0):
            for jq in range(NQB):
                cpsA = bpool.tile([VW, QB], f32, tag="b1", name="cpsA")
                cpsB = bpool.tile([VW, QB], f32, tag="b1", name="cpsB")
                nt = 4 * (jq + 1)          # valid kv tiles

                def pv(t, att):
                    dq = max(0, t * P - jq * QB)
                    for rh, cps in ((0, cpsA), (1, cpsB)):
                        h = 2 * jh + rh
                        nc.tensor.matmul(
                            cps[:, dq:QB],
                            vsb[:, (h * NKV + t) * VW: (h * NKV + t + 1) * VW],
                            att[:, rh * QB + dq:(rh + 1) * QB],
                            start=(t == 0),
                            stop=(t == nt - 1),
                            skip_group_check=True)

                pending = []
                for t in range(nt):
                    sps = spool.tile([P, 2 * QB], f32, tag="sp", name="sps")
                    att = apool.tile([P, 2 * QB], bf16, tag="att", name="att")
                    dq = max(0, t * P - jq * QB)  # causal left-trim
                    for rh in range(2):
                        r0 = rh * HD
                        nc.tensor.matmul(
                            sps[:, rh * QB + dq:(rh + 1) * QB],
                            kT[r0:r0 + HD, jh * S + t * P: jh * S + (t + 1) * P],
                            qT[r0:r0 + HD,
                               jh * S + jq * QB + dq: jh * S + (jq + 1) * QB],
                            start=True, stop=True)
                    nc.scalar.activation(att[:], sps[:], Exp, scale=0.125)
                    if t >= 4 * jq:
                        # zero the upper triangle of the diagonal 128x128
                        # block in both halves: one op, free dims (2, 128)
                        attv = att.rearrange("p (h c) -> p h c", h=2)
                        nc.vector.tensor_mul(
                            attv[:, :, dq:dq + P], attv[:, :, dq:dq + P],
                            msk.rearrange("p (h c) -> p h c", h=2))
                    # PV runs 2 kv-tiles behind scores so PE never waits on
                    # exp (ACT) + mask (DVE) latency
                    pending.append((t, att))
                    if len(pending) > 2:
                        pv(*pending.pop(0))
                for pe_args in pending:
                    pv(*pe_args)
                # normalize: ctxT = cps[0:64] * (1 / denom row), write into cx
                for rh, cps in ((0, cpsA), (1, cpsB)):
                    r0 = rh * HD
                    rec1 = work.tile([1, QB], mybir.dt.float32, tag="rec", name="rec1")
                    nc.vector.reciprocal(rec1[:], cps[HD:HD + 1, :])
                    recb = work.tile([HD, QB], mybir.dt.float32, tag="recb", name="recb")
                    nc.gpsimd.partition_broadcast(recb[:], rec1[:])
                    nc.vector.tensor_mul(
                        cx[r0:r0 + HD, jh * S + jq * QB: jh * S + (jq + 1) * QB],
                        cps[0:HD, :], recb[:])

        # ---- Phase D: out projection ---------------------------------
        # round-robin psum over both pools (sp slots are free now) and
        # alternate the psum->sbuf copy between DVE and ACT so PE never
        # waits on a drain
        for st in range(NKV if "d" in parts else 0):
            for eb in range(2):
                i = st * 2 + eb
                if i % 2 == 0:
                    ops = bpool.tile([P, QB], f32, tag="b1", name="ops")
                else:
                    ops = spool.tile([P, QB], f32, tag="sp", name="ops2")
                for fc in range(NJ):
                    nc.tensor.matmul(
                        ops[:],
                        cx[:, fc * S + st * P: fc * S + (st + 1) * P],
                        wo[:, fc * D + eb * QB: fc * D + (eb + 1) * QB],
                        start=(fc == 0), stop=(fc == NJ - 1))
                ob = opool.tile([P, QB], f32, tag="ob", name="ob")
                if i % 2 == 0:
                    nc.vector.tensor_copy(ob[:], ops[:])
                else:
                    nc.scalar.copy(ob[:], ops[:])
                nc.sync.dma_start(
                    out_d[st * P:(st + 1) * P, eb * QB:(eb + 1) * QB], ob[:])

        loop_cm.__exit__(None, None, None)

    nc.compile()
    return nc


def _chunk_rows(a, np_rows):
    """(n*P, F) -> (P, n*F) with chunk i of rows at free offset i*F."""
    n = a.shape[0] // np_rows
    return np.ascontiguousarray(
        a.reshape(n, np_rows, a.shape[1]).transpose(1, 0, 2).reshape(np_rows, -1))


def _host_tables():
    inv = 1.0 / (ROPE_BASE ** (np.arange(0, HD, 2, dtype=np.float64) / HD))
    t = np.arange(S, dtype=np.float64)
    fr = np.outer(inv, t)                      # (32, S)
    cosw = np.cos(fr)
    sinw = np.sin(fr)
    cosT = np.tile(cosw, (4, 1))               # rows: 4 x 32 pattern
    sinT = np.tile(sinw, (4, 1))               # sign lives in rmat
    # 0/1 upper-triangle (keep q >= kv) mask for diagonal blocks, doubled
    r = np.arange(P)[:, None]
    c = np.arange(P)[None, :]
    tri = (c >= r).astype(np.float32)
    m = np.concatenate([tri, tri], axis=1)
    # rotate-half matrix: out[p] = sum_r rm[r, p] * in[r]
    rmat = np.zeros((P, P), np.float32)
    for b in range(2):
        for i in range(32):
            p = b * 64 + i
            rmat[p + 32, p] = -1.0             # out[p] = -in[p+32]
            rmat[p, p + 32] = 1.0              # out[p+32] = in[p]
    return (cosT.astype(BF), sinT.astype(BF), m.astype(BF), rmat.astype(BF))


def kernel(x, Wqkv, Wout, mask=None, **_unused):
    x = np.asarray(x, dtype=np.float32)
    Wqkv = np.asarray(Wqkv, dtype=np.float32)
    Wout = np.asarray(Wout, dtype=np.float32)
    B = x.shape[0]

    cosT, sinT, mskv, rmat = _host_tables()

    in_maps = []
    for core in range(NCORES):
        b, hg = core // 2, core % 2
        sl = slice(hg * DQ, hg * DQ + DQ)
        xT = np.ascontiguousarray(x[b].T).astype(BF)       # (1024, 2048)
        in_maps.append({
            "xT": _chunk_rows(xT, P),
            "wq": _chunk_rows(Wqkv[:, 0 * D + hg * DQ: 0 * D + hg * DQ + DQ].astype(BF), P),
            "wk": _chunk_rows(Wqkv[:, 1 * D + hg * DQ: 1 * D + hg * DQ + DQ].astype(BF), P),
            "wv": _chunk_rows(Wqkv[:, 2 * D + hg * DQ: 2 * D + hg * DQ + DQ].astype(BF), P),
            "wo": _chunk_rows(Wout[sl, :].astype(BF), P),
            "cosT": cosT, "sinT": sinT, "msk": mskv, "rm": rmat,
        })

    if "nc" not in _CACHE:
        _CACHE["nc"] = _build_nc()
    nc = _CACHE["nc"]

    from concourse.bass_utils import run_bass_kernel_spmd
    res = run_bass_kernel_spmd(nc, in_maps, list(range(NCORES))).results

    out = np.zeros((B, S, D), np.float32)
    for core in range(NCORES):
        out[core // 2] += res[core]["outp"]
    return out
